# revision 53
# baseline (speedup 1.0000x reference)
"""Windowed spatial MSA with precomputed attention — TRN2 Bass kernel (bf16).

Math per window n (64 tokens, C=256):
    out_n = attn_n @ (x_n @ Wv) @ Wo + bo
         == attn_n @ x_n @ W + bo,   W = Wv @ Wo   (folded on host)

Device pipeline, bf16 in / fp32 PSUM / bf16 out, per 128-token tile
(= one pair of windows packed on the 128 partitions):
  step1: Y[tok,Cout] = sum_ci (X^T chunk).T @ W chunk    2 MMs, N=256
         lhsT = X^T [c_lo, tok] (stationary, FWL), rhs = W [c_lo, Cout]
  step2: O[tok,Cout] = A_w @ Y_w per window              2 MMs, N=256
         lhsT = A^T [k, m] 64x64 per window; the two windows of the
         pair run in disjoint PE quadrants (rows/cols 0-63 vs 64-127)
         and write disjoint partition halves of one PSUM bank.

Host pre-permutes all operands so every DMA is a flat 128-partition copy;
bf16 halves both the DMA bytes and the PE streaming cost (fp32 is
4 cycles/row on the PE and gets no fast-weight-load).

Sharding: data-parallel; each core owns 16 of the 128 window-row-groups
(256 token-tiles of 128 tokens), processed as 8 batches of 32 tiles.
"""

import numpy as np
import ml_dtypes

import concourse.bacc as bacc
import concourse.mybir as mybir
from concourse.tile import TileContext
from concourse.bass_utils import run_bass_kernel_spmd

N_CORES = 8
B, H, WD, C = 4, 256, 256, 256
WS = 8
SEQ = WS * WS  # 64 tokens per window
NWJ = WD // WS  # 32 windows per row-group
GROUPS = B * (H // WS)  # 128 row-groups of 8 pixel rows
G_PER_CORE = GROUPS // N_CORES  # 16
GPAIRS = G_PER_CORE // 2  # 8 group-pairs per core
NB = NWJ  # token-tiles (window-pairs) per batch = 32
NBATCH = GPAIRS  # 8 batches per core; batch == group-pair
CH = C // 128  # 2 contraction chunks of 128

F32 = mybir.dt.float32
BF16 = mybir.dt.bfloat16
NPBF16 = np.dtype(ml_dtypes.bfloat16)


def build_nc(nb=NB, nbatch=NBATCH, num_devices=N_CORES, x_split=2, depth=2):
    """Per-core Bass module (SPMD: all cores run the same program).

    DRAM layouts (host pre-permuted, all bf16):
      xt:  [nbatch, 128, CH*nb*128]  partitions=c_lo, free=(ci, t, lane)
      at:  [nbatch, 128, nb*SEQ]     partitions=(g2,k), free=(t, m)
      w:   [128, CH*C]               partitions=c_lo, free=(ci, Cout)
      out: [nbatch, 128, nb*C]       partitions=lane, free=(t, Cout)
    where lane = g2*64 + p*8 + q and t indexes window-pairs in the batch.
    """
    nc = bacc.Bacc(
        "TRN2", target_bir_lowering=False, debug=False, num_devices=num_devices
    )
    xt = nc.dram_tensor("xt", [nbatch, 128, CH * nb * 128], BF16, kind="ExternalInput")
    at = nc.dram_tensor("at", [nbatch, 128, nb, 2 * SEQ], BF16, kind="ExternalInput")
    w = nc.dram_tensor("w", [128, CH * C], BF16, kind="ExternalInput")
    out = nc.dram_tensor("out", [nbatch, 128, nb * C], BF16, kind="ExternalOutput")

    NPAIRS_B = nb // 2  # tile-pairs per batch; PSUM/copies batched per pair

    with TileContext(nc) as tc:
        with (
            tc.tile_pool(name="wpool", bufs=1) as wpool,
            tc.tile_pool(name="xpool", bufs=4) as xpool,
            tc.tile_pool(name="apool", bufs=3) as apool,
            tc.tile_pool(name="ypool", bufs=depth + 2) as ypool,
            tc.tile_pool(name="opool", bufs=4) as opool,
            tc.tile_pool(name="ypsum", bufs=depth + 2, space="PSUM") as ypsum,
            tc.tile_pool(name="opsum", bufs=4, space="PSUM") as opsum,
        ):
            w_sb = wpool.tile([128, CH * C], BF16)
            nc.sync.dma_start(w_sb[:], w[:])

            def step2(p, y_sb, a_sb, o_sb, nbi):
                """Attention matmuls + output copy for tile-pair p (software-
                pipelined `depth` pairs behind step1 so weight loads and the
                y-copy latency hide under running matmuls). lhsT is a 128x128
                block-diagonal A^T covering both windows of the tile."""
                pp = p % NPAIRS_B
                o_ps = opsum.tile([128, 2 * C], F32, tag="ops", name="o_ps")
                for half in range(2):
                    t = 2 * pp + half
                    nc.tensor.matmul(
                        o_ps[:, half * C : (half + 1) * C],
                        lhsT=a_sb[:, t, :],
                        rhs=y_sb[:, half * C : (half + 1) * C],
                        start=True,
                        stop=True,
                    )
                if p % 2 == 0:
                    nc.vector.tensor_copy(
                        o_sb[:, pp * 2 * C : (pp + 1) * 2 * C], o_ps[:]
                    )
                else:
                    nc.scalar.copy(o_sb[:, pp * 2 * C : (pp + 1) * 2 * C], o_ps[:])
                if pp == NPAIRS_B - 1:
                    nc.sync.dma_start(out[nbi], o_sb[:])

            pending = []
            x_sb = a_sb = o_sb = None
            for p in range(nbatch * NPAIRS_B):
                nbi, pp = divmod(p, NPAIRS_B)
                if pp == 0:
                    x_sb = xpool.tile([128, CH * nb * 128], BF16, tag="x", name="x_sb")
                    xw = CH * nb * 128 // x_split
                    for s in range(x_split):
                        nc.sync.dma_start(
                            x_sb[:, s * xw : (s + 1) * xw],
                            xt[nbi, :, s * xw : (s + 1) * xw],
                        )
                    # Host bakes the block-diagonal (zeros included) so this is
                    # one contiguous DMA with 8KB-per-partition runs; a strided
                    # DMA here shatters into 128-byte descriptors and swamps
                    # the DMA rings.
                    a_sb = apool.tile([128, nb, 2 * SEQ], BF16, tag="a", name="a_sb")
                    nc.sync.dma_start(a_sb[:], at[nbi])
                    o_sb = opool.tile([128, nb * C], BF16, tag="o", name="o_sb")

                y_ps = ypsum.tile([128, 2 * C], F32, tag="yps", name="y_ps")
                for half in range(2):
                    t = 2 * pp + half
                    for ci in range(CH):
                        nc.tensor.matmul(
                            y_ps[:, half * C : (half + 1) * C],
                            lhsT=x_sb[:, (ci * nb + t) * 128 : (ci * nb + t + 1) * 128],
                            rhs=w_sb[:, ci * C : (ci + 1) * C],
                            start=(ci == 0),
                            stop=(ci == CH - 1),
                        )
                y_sb = ypool.tile([128, 2 * C], BF16, tag="y", name="y_sb")
                if p % 2 == 0:
                    nc.scalar.copy(y_sb[:], y_ps[:])
                else:
                    nc.vector.tensor_copy(y_sb[:], y_ps[:])

                pending.append((p, y_sb, a_sb, o_sb, nbi))
                if len(pending) > depth:
                    step2(*pending.pop(0))
            for args in pending:
                step2(*args)
    nc.compile()
    return nc


_NC_CACHE = {}


def get_nc(key="bf16"):
    if key not in _NC_CACHE:
        _NC_CACHE[key] = build_nc()
    return _NC_CACHE[key]


def make_in_maps(x, attn, Wv, Wo):
    x = np.asarray(x, dtype=np.float32)
    attn = np.asarray(attn, dtype=np.float32)
    W = np.asarray(Wv, dtype=np.float32) @ np.asarray(Wo, dtype=np.float32)
    wprep = np.ascontiguousarray(
        W.reshape(CH, 128, C).transpose(1, 0, 2).reshape(128, CH * C).astype(NPBF16)
    )
    # x: (B,H,W,C) -> [core, gp, g2, p, j, q, ci, c_lo] -> [core, gp, c_lo, ci, j, g2, p, q]
    xb = x.astype(NPBF16)
    xg = xb.reshape(N_CORES, GPAIRS, 2, WS, NWJ, WS, CH, 128)
    xg = np.ascontiguousarray(xg.transpose(0, 1, 7, 6, 4, 2, 3, 5))
    xg = xg.reshape(N_CORES, NBATCH, 128, CH * NB * 128)
    # attn: (NW,1,m,k) -> A^T blocks on the diagonal of a per-pair 128x128
    # (partitions (g2,k), free (j, g2*64+m)); off-diagonal zeros baked here.
    ab = attn.astype(NPBF16)
    av = ab.reshape(N_CORES, GPAIRS, 2, NWJ, SEQ, SEQ).transpose(0, 1, 2, 5, 3, 4)
    ag = np.zeros((N_CORES, GPAIRS, 2, SEQ, NWJ, 2, SEQ), dtype=NPBF16)
    for g2 in range(2):
        ag[:, :, g2, :, :, g2, :] = av[:, :, g2]
    ag = ag.reshape(N_CORES, NBATCH, 128, NB, 2 * SEQ)
    return [
        {"xt": xg[cid], "at": ag[cid], "w": wprep} for cid in range(N_CORES)
    ]


def assemble_out(results, bo):
    out = np.empty((GROUPS, WS, NWJ, WS, C), dtype=np.float32)
    for cid in range(N_CORES):
        r = np.asarray(results[cid]["out"], dtype=np.float32)
        # [gp, (g2,p,q), (j,Cout)] -> [gp, g2, p, j, q, c] -> [16 groups, p, j, q, c]
        r = r.reshape(GPAIRS, 2, WS, WS, NWJ, C).transpose(0, 1, 2, 4, 3, 5)
        out[cid * G_PER_CORE : (cid + 1) * G_PER_CORE] = r.reshape(
            G_PER_CORE, WS, NWJ, WS, C
        )
    out = out.reshape(B, H, WD, C)
    bo = np.asarray(bo, dtype=np.float32)
    if np.any(bo):
        out = out + bo
    return out


def run(x, attn, Wv, Wo, bo, dt_mm=None, **spmd_kwargs):
    nc = get_nc()
    in_maps = make_in_maps(x, attn, Wv, Wo)
    res = run_bass_kernel_spmd(nc, in_maps, core_ids=list(range(N_CORES)), **spmd_kwargs)
    return assemble_out(res.results, bo), res


def kernel(x, attn, Wv, Wo, bo):
    out, _ = run(x, attn, Wv, Wo, bo)
    return out


# revision 55
# speedup vs baseline: 1.1064x; 1.1064x over previous
"""Windowed spatial MSA with precomputed attention — TRN2 Bass kernel (bf16).

Math per window n (64 tokens, C=256):
    out_n = attn_n @ (x_n @ Wv) @ Wo + bo
         == attn_n @ x_n @ W + bo,   W = Wv @ Wo   (folded on host)

Device pipeline, bf16 in / fp32 PSUM / bf16 out, per 128-token tile
(= one pair of windows packed on the 128 partitions):
  step1: Y[tok,Cout] = sum_ci (X^T chunk).T @ W chunk    2 MMs, N=256
         lhsT = X^T [c_lo, tok] (stationary), rhs = W [c_lo, Cout]
  step2: O[tok,Cout] = A_w @ Y_w, both windows at once   1 MM, N=256
         lhsT = 128x128 block-diagonal A^T (host-padded zeros), so the
         pair needs a single stationary operand per half instead of two
         concurrent 64x64 quadrant loads that would pin both PE weight
         buffers and expose the next weight load.

Step2 runs `depth` pairs behind step1 (software pipeline), so every
LDWEIGHTS overlaps a running matmul and the PSUM->SBUF copy latency is
hidden; y/o copies are batched per pair ([128,512] = one PSUM bank) and
alternate between the Scalar and Vector engines — the only two engines
with PSUM access. Host pre-permutes all operands so every DMA is a flat
contiguous 128-partition copy (strided DMAs shatter into 128B
descriptors and swamp the rings); bf16 halves both the DMA bytes and
the PE streaming cost (fp32 is 4 cycles/row and gets no fast weight
load).

Sharding: data-parallel; each core owns 16 of the 128 window-row-groups
(256 token-tiles of 128 tokens), processed as 8 batches of 32 tiles.
Measured on 8 trn2 cores: ~131-138 us vs 457 us for the fp32 baseline.
"""

import numpy as np
import ml_dtypes

import concourse.bacc as bacc
import concourse.mybir as mybir
from concourse.tile import TileContext
from concourse.bass_utils import run_bass_kernel_spmd

N_CORES = 8
B, H, WD, C = 4, 256, 256, 256
WS = 8
SEQ = WS * WS  # 64 tokens per window
NWJ = WD // WS  # 32 windows per row-group
GROUPS = B * (H // WS)  # 128 row-groups of 8 pixel rows
G_PER_CORE = GROUPS // N_CORES  # 16
GPAIRS = G_PER_CORE // 2  # 8 group-pairs per core
NB = NWJ  # token-tiles (window-pairs) per batch = 32
NBATCH = GPAIRS  # 8 batches per core; batch == group-pair
CH = C // 128  # 2 contraction chunks of 128

F32 = mybir.dt.float32
BF16 = mybir.dt.bfloat16
NPBF16 = np.dtype(ml_dtypes.bfloat16)


def build_nc(nb=NB, nbatch=NBATCH, num_devices=N_CORES, x_split=2, depth=2):
    """Per-core Bass module (SPMD: all cores run the same program).

    DRAM layouts (host pre-permuted, all bf16):
      xt:  [nbatch, 128, CH*nb*128]  partitions=c_lo, free=(ci, t, lane)
      at:  [nbatch, 128, nb*SEQ]     partitions=(g2,k), free=(t, m)
      w:   [128, CH*C]               partitions=c_lo, free=(ci, Cout)
      out: [nbatch, 128, nb*C]       partitions=lane, free=(t, Cout)
    where lane = g2*64 + p*8 + q and t indexes window-pairs in the batch.
    """
    nc = bacc.Bacc(
        "TRN2", target_bir_lowering=False, debug=False, num_devices=num_devices
    )
    xt = nc.dram_tensor("xt", [nbatch, 128, CH * nb * 128], BF16, kind="ExternalInput")
    at = nc.dram_tensor("at", [nbatch, 128, nb, 2 * SEQ], BF16, kind="ExternalInput")
    w = nc.dram_tensor("w", [128, CH * C], BF16, kind="ExternalInput")
    out = nc.dram_tensor("out", [nbatch, 128, nb * C], BF16, kind="ExternalOutput")

    NPAIRS_B = nb // 2  # tile-pairs per batch; PSUM/copies batched per pair

    with TileContext(nc) as tc:
        with (
            tc.tile_pool(name="wpool", bufs=1) as wpool,
            tc.tile_pool(name="xpool", bufs=4) as xpool,
            tc.tile_pool(name="apool", bufs=3) as apool,
            tc.tile_pool(name="ypool", bufs=depth + 2) as ypool,
            tc.tile_pool(name="opool", bufs=4) as opool,
            tc.tile_pool(name="ypsum", bufs=depth + 2, space="PSUM") as ypsum,
            tc.tile_pool(name="opsum", bufs=3, space="PSUM") as opsum,
        ):
            w_sb = wpool.tile([128, CH * C], BF16)
            nc.sync.dma_start(w_sb[:], w[:])

            def step2(p, y_sb, a_sb, o_sb, nbi):
                """Attention matmuls + output copy for tile-pair p (software-
                pipelined `depth` pairs behind step1 so weight loads and the
                y-copy latency hide under running matmuls). lhsT is a 128x128
                block-diagonal A^T covering both windows of the tile."""
                pp = p % NPAIRS_B
                o_ps = opsum.tile([128, 2 * C], F32, tag="ops", name="o_ps")
                for half in range(2):
                    t = 2 * pp + half
                    nc.tensor.matmul(
                        o_ps[:, half * C : (half + 1) * C],
                        lhsT=a_sb[:, t, :],
                        rhs=y_sb[:, half * C : (half + 1) * C],
                        start=True,
                        stop=True,
                    )
                if p % 2 == 0:
                    nc.vector.tensor_copy(
                        o_sb[:, pp * 2 * C : (pp + 1) * 2 * C], o_ps[:]
                    )
                else:
                    nc.scalar.copy(o_sb[:, pp * 2 * C : (pp + 1) * 2 * C], o_ps[:])
                if pp == NPAIRS_B - 1:
                    nc.sync.dma_start(out[nbi], o_sb[:])

            pending = []
            x_sb = a_sb = o_sb = None
            for p in range(nbatch * NPAIRS_B):
                nbi, pp = divmod(p, NPAIRS_B)
                if pp == 0:
                    x_sb = xpool.tile([128, CH * nb * 128], BF16, tag="x", name="x_sb")
                    xw = CH * nb * 128 // x_split
                    for s in range(x_split):
                        nc.sync.dma_start(
                            x_sb[:, s * xw : (s + 1) * xw],
                            xt[nbi, :, s * xw : (s + 1) * xw],
                        )
                    # Host bakes the block-diagonal (zeros included) so this is
                    # one contiguous DMA with 8KB-per-partition runs; a strided
                    # DMA here shatters into 128-byte descriptors and swamps
                    # the DMA rings.
                    a_sb = apool.tile([128, nb, 2 * SEQ], BF16, tag="a", name="a_sb")
                    nc.sync.dma_start(a_sb[:], at[nbi])
                    o_sb = opool.tile([128, nb * C], BF16, tag="o", name="o_sb")

                y_ps = ypsum.tile([128, 2 * C], F32, tag="yps", name="y_ps")
                for half in range(2):
                    t = 2 * pp + half
                    for ci in range(CH):
                        nc.tensor.matmul(
                            y_ps[:, half * C : (half + 1) * C],
                            lhsT=x_sb[:, (ci * nb + t) * 128 : (ci * nb + t + 1) * 128],
                            rhs=w_sb[:, ci * C : (ci + 1) * C],
                            start=(ci == 0),
                            stop=(ci == CH - 1),
                        )
                y_sb = ypool.tile([128, 2 * C], BF16, tag="y", name="y_sb")
                if p % 2 == 0:
                    nc.scalar.copy(y_sb[:], y_ps[:])
                else:
                    nc.vector.tensor_copy(y_sb[:], y_ps[:])

                pending.append((p, y_sb, a_sb, o_sb, nbi))
                if len(pending) > depth:
                    step2(*pending.pop(0))
            for args in pending:
                step2(*args)
    nc.compile()
    return nc


_NC_CACHE = {}


def get_nc(key="bf16"):
    if key not in _NC_CACHE:
        _NC_CACHE[key] = build_nc()
    return _NC_CACHE[key]


def make_in_maps(x, attn, Wv, Wo):
    x = np.asarray(x, dtype=np.float32)
    attn = np.asarray(attn, dtype=np.float32)
    W = np.asarray(Wv, dtype=np.float32) @ np.asarray(Wo, dtype=np.float32)
    wprep = np.ascontiguousarray(
        W.reshape(CH, 128, C).transpose(1, 0, 2).reshape(128, CH * C).astype(NPBF16)
    )
    # x: (B,H,W,C) -> [core, gp, g2, p, j, q, ci, c_lo] -> [core, gp, c_lo, ci, j, g2, p, q]
    xb = x.astype(NPBF16)
    xg = xb.reshape(N_CORES, GPAIRS, 2, WS, NWJ, WS, CH, 128)
    xg = np.ascontiguousarray(xg.transpose(0, 1, 7, 6, 4, 2, 3, 5))
    xg = xg.reshape(N_CORES, NBATCH, 128, CH * NB * 128)
    # attn: (NW,1,m,k) -> A^T blocks on the diagonal of a per-pair 128x128
    # (partitions (g2,k), free (j, g2*64+m)); off-diagonal zeros baked here.
    ab = attn.astype(NPBF16)
    av = ab.reshape(N_CORES, GPAIRS, 2, NWJ, SEQ, SEQ).transpose(0, 1, 2, 5, 3, 4)
    ag = np.zeros((N_CORES, GPAIRS, 2, SEQ, NWJ, 2, SEQ), dtype=NPBF16)
    for g2 in range(2):
        ag[:, :, g2, :, :, g2, :] = av[:, :, g2]
    ag = ag.reshape(N_CORES, NBATCH, 128, NB, 2 * SEQ)
    return [
        {"xt": xg[cid], "at": ag[cid], "w": wprep} for cid in range(N_CORES)
    ]


def assemble_out(results, bo):
    out = np.empty((GROUPS, WS, NWJ, WS, C), dtype=np.float32)
    for cid in range(N_CORES):
        r = np.asarray(results[cid]["out"], dtype=np.float32)
        # [gp, (g2,p,q), (j,Cout)] -> [gp, g2, p, j, q, c] -> [16 groups, p, j, q, c]
        r = r.reshape(GPAIRS, 2, WS, WS, NWJ, C).transpose(0, 1, 2, 4, 3, 5)
        out[cid * G_PER_CORE : (cid + 1) * G_PER_CORE] = r.reshape(
            G_PER_CORE, WS, NWJ, WS, C
        )
    out = out.reshape(B, H, WD, C)
    bo = np.asarray(bo, dtype=np.float32)
    if np.any(bo):
        out = out + bo
    return out


def run(x, attn, Wv, Wo, bo, dt_mm=None, **spmd_kwargs):
    nc = get_nc()
    in_maps = make_in_maps(x, attn, Wv, Wo)
    res = run_bass_kernel_spmd(nc, in_maps, core_ids=list(range(N_CORES)), **spmd_kwargs)
    return assemble_out(res.results, bo), res


def kernel(x, attn, Wv, Wo, bo):
    out, _ = run(x, attn, Wv, Wo, bo)
    return out


# revision 59
# speedup vs baseline: 1.2392x; 1.1200x over previous
"""Windowed spatial MSA with precomputed attention — TRN2 Bass kernel (bf16).

Math per window n (64 tokens, C=256):
    out_n = attn_n @ (x_n @ Wv) @ Wo + bo
         == attn_n @ x_n @ W + bo,   W = Wv @ Wo   (folded on host)

Device pipeline, bf16 in / fp32 PSUM / bf16 out, per 128-token tile
(= one pair of windows packed on the 128 partitions):
  step1: Y[tok,Cout] = sum_ci (X^T chunk).T @ W chunk    2 MMs, N=256
         lhsT = X^T [c_lo, tok] (stationary), rhs = W [c_lo, Cout]
  step2: O[tok,Cout] = A_w @ Y_w, both windows at once   1 MM, N=256
         lhsT = 128x128 block-diagonal A^T (host-padded zeros), so the
         pair needs a single stationary operand per half instead of two
         concurrent 64x64 quadrant loads that would pin both PE weight
         buffers and expose the next weight load.

Step2 runs `depth` pairs behind step1 (software pipeline), so every
LDWEIGHTS overlaps a running matmul and the PSUM->SBUF copy latency is
hidden; y/o copies are batched per pair ([128,512] = one PSUM bank) and
alternate between the Scalar and Vector engines — the only two engines
with PSUM access. Host pre-permutes all operands so every DMA is a flat
contiguous 128-partition copy (strided DMAs shatter into 128B
descriptors and swamp the rings); bf16 halves both the DMA bytes and
the PE streaming cost (fp32 is 4 cycles/row and gets no fast weight
load).

Sharding: data-parallel; each core owns 16 of the 128 window-row-groups
(256 token-tiles of 128 tokens), processed as 8 batches of 32 tiles.
Measured on 8 trn2 cores: ~131-138 us vs 457 us for the fp32 baseline.
"""

import numpy as np
import ml_dtypes

import concourse.bacc as bacc
import concourse.mybir as mybir
from concourse.tile import TileContext
from concourse.bass_utils import run_bass_kernel_spmd

N_CORES = 8
B, H, WD, C = 4, 256, 256, 256
WS = 8
SEQ = WS * WS  # 64 tokens per window
NWJ = WD // WS  # 32 windows per row-group
GROUPS = B * (H // WS)  # 128 row-groups of 8 pixel rows
G_PER_CORE = GROUPS // N_CORES  # 16
GPAIRS = G_PER_CORE // 2  # 8 group-pairs per core
NB = NWJ  # token-tiles (window-pairs) per batch = 32
NBATCH = GPAIRS  # 8 batches per core; batch == group-pair
CH = C // 128  # 2 contraction chunks of 128

F32 = mybir.dt.float32
BF16 = mybir.dt.bfloat16
NPBF16 = np.dtype(ml_dtypes.bfloat16)


def build_nc(nb=NB, nbatch=NBATCH, num_devices=N_CORES, x_split=4, depth=2):
    """Per-core Bass module (SPMD: all cores run the same program).

    DRAM layouts (host pre-permuted, all bf16):
      xt:  [nbatch, 128, CH*nb*128]  partitions=c_lo, free=(ci, t, lane)
      at:  [nbatch, 128, nb*SEQ]     partitions=(g2,k), free=(t, m)
      w:   [128, CH*C]               partitions=c_lo, free=(ci, Cout)
      out: [nbatch, 128, nb*C]       partitions=lane, free=(t, Cout)
    where lane = g2*64 + p*8 + q and t indexes window-pairs in the batch.
    """
    nc = bacc.Bacc(
        "TRN2", target_bir_lowering=False, debug=False, num_devices=num_devices
    )
    xt = nc.dram_tensor("xt", [nbatch, 128, CH * nb * 128], BF16, kind="ExternalInput")
    at = nc.dram_tensor("at", [nbatch, 128, nb, 2 * SEQ], BF16, kind="ExternalInput")
    w = nc.dram_tensor("w", [128, CH * C], BF16, kind="ExternalInput")
    out = nc.dram_tensor("out", [nbatch, 128, nb * C], BF16, kind="ExternalOutput")

    NPAIRS_B = nb // 2  # tile-pairs per batch; PSUM/copies batched per pair

    with TileContext(nc) as tc:
        with (
            tc.tile_pool(name="wpool", bufs=1) as wpool,
            tc.tile_pool(name="xpool", bufs=4) as xpool,
            tc.tile_pool(name="apool", bufs=3) as apool,
            tc.tile_pool(name="ypool", bufs=depth + 2) as ypool,
            tc.tile_pool(name="opool", bufs=4) as opool,
            tc.tile_pool(name="ypsum", bufs=depth + 2, space="PSUM") as ypsum,
            tc.tile_pool(name="opsum", bufs=3, space="PSUM") as opsum,
        ):
            w_sb = wpool.tile([128, CH * C], BF16)
            nc.sync.dma_start(w_sb[:], w[:])

            def step2(p, y_sb, a_sb, o_sb, nbi):
                """Attention matmuls + output copy for tile-pair p (software-
                pipelined `depth` pairs behind step1 so weight loads and the
                y-copy latency hide under running matmuls). lhsT is a 128x128
                block-diagonal A^T covering both windows of the tile."""
                pp = p % NPAIRS_B
                o_ps = opsum.tile([128, 2 * C], F32, tag="ops", name="o_ps")
                for half in range(2):
                    t = 2 * pp + half
                    nc.tensor.matmul(
                        o_ps[:, half * C : (half + 1) * C],
                        lhsT=a_sb[:, t, :],
                        rhs=y_sb[:, half * C : (half + 1) * C],
                        start=True,
                        stop=True,
                    )
                if p % 2 == 0:
                    nc.vector.tensor_copy(
                        o_sb[:, pp * 2 * C : (pp + 1) * 2 * C], o_ps[:]
                    )
                else:
                    nc.scalar.copy(o_sb[:, pp * 2 * C : (pp + 1) * 2 * C], o_ps[:])
                if nbi == nbatch - 1:
                    # Last batch: drain the output in quarters as pairs finish
                    # — there are no later input DMAs behind these on the sync
                    # ring to block, and it cuts the serial end-of-kernel tail.
                    if pp % 4 == 3:
                        nc.sync.dma_start(
                            out[nbi, :, (pp - 3) * 2 * C : (pp + 1) * 2 * C],
                            o_sb[:, (pp - 3) * 2 * C : (pp + 1) * 2 * C],
                        )
                elif pp == NPAIRS_B - 1:
                    nc.sync.dma_start(out[nbi], o_sb[:])

            pending = []
            x_sb = a_sb = o_sb = None
            for p in range(nbatch * NPAIRS_B):
                nbi, pp = divmod(p, NPAIRS_B)
                if pp == 0:
                    x_sb = xpool.tile([128, CH * nb * 128], BF16, tag="x", name="x_sb")
                    xw = CH * nb * 128 // x_split
                    for s in range(x_split):
                        nc.sync.dma_start(
                            x_sb[:, s * xw : (s + 1) * xw],
                            xt[nbi, :, s * xw : (s + 1) * xw],
                        )
                    # Host bakes the block-diagonal (zeros included) so this is
                    # one contiguous DMA with 8KB-per-partition runs; a strided
                    # DMA here shatters into 128-byte descriptors and swamps
                    # the DMA rings.
                    a_sb = apool.tile([128, nb, 2 * SEQ], BF16, tag="a", name="a_sb")
                    nc.sync.dma_start(a_sb[:], at[nbi])
                    o_sb = opool.tile([128, nb * C], BF16, tag="o", name="o_sb")

                y_ps = ypsum.tile([128, 2 * C], F32, tag="yps", name="y_ps")
                for half in range(2):
                    t = 2 * pp + half
                    for ci in range(CH):
                        nc.tensor.matmul(
                            y_ps[:, half * C : (half + 1) * C],
                            lhsT=x_sb[:, (t * CH + ci) * 128 : (t * CH + ci + 1) * 128],
                            rhs=w_sb[:, ci * C : (ci + 1) * C],
                            start=(ci == 0),
                            stop=(ci == CH - 1),
                        )
                y_sb = ypool.tile([128, 2 * C], BF16, tag="y", name="y_sb")
                if p % 2 == 0:
                    nc.scalar.copy(y_sb[:], y_ps[:])
                else:
                    nc.vector.tensor_copy(y_sb[:], y_ps[:])

                pending.append((p, y_sb, a_sb, o_sb, nbi))
                if len(pending) > depth:
                    step2(*pending.pop(0))
            for args in pending:
                step2(*args)
    nc.compile()
    return nc


_NC_CACHE = {}


def get_nc(key="bf16"):
    if key not in _NC_CACHE:
        _NC_CACHE[key] = build_nc()
    return _NC_CACHE[key]


def make_in_maps(x, attn, Wv, Wo):
    x = np.asarray(x, dtype=np.float32)
    attn = np.asarray(attn, dtype=np.float32)
    W = np.asarray(Wv, dtype=np.float32) @ np.asarray(Wo, dtype=np.float32)
    wprep = np.ascontiguousarray(
        W.reshape(CH, 128, C).transpose(1, 0, 2).reshape(128, CH * C).astype(NPBF16)
    )
    # x: (B,H,W,C) -> [core, gp, g2, p, j, q, ci, c_lo] -> [core, gp, c_lo, j, ci, g2, p, q]
    # (t outer, ci inner: each quarter of the batch DMA is self-contained, so
    # compute starts after the first 0.5MB instead of the full 2MB)
    xb = x.astype(NPBF16)
    xg = xb.reshape(N_CORES, GPAIRS, 2, WS, NWJ, WS, CH, 128)
    xg = np.ascontiguousarray(xg.transpose(0, 1, 7, 4, 6, 2, 3, 5))
    xg = xg.reshape(N_CORES, NBATCH, 128, CH * NB * 128)
    # attn: (NW,1,m,k) -> A^T blocks on the diagonal of a per-pair 128x128
    # (partitions (g2,k), free (j, g2*64+m)); off-diagonal zeros baked here.
    ab = attn.astype(NPBF16)
    av = ab.reshape(N_CORES, GPAIRS, 2, NWJ, SEQ, SEQ).transpose(0, 1, 2, 5, 3, 4)
    ag = np.zeros((N_CORES, GPAIRS, 2, SEQ, NWJ, 2, SEQ), dtype=NPBF16)
    for g2 in range(2):
        ag[:, :, g2, :, :, g2, :] = av[:, :, g2]
    ag = ag.reshape(N_CORES, NBATCH, 128, NB, 2 * SEQ)
    return [
        {"xt": xg[cid], "at": ag[cid], "w": wprep} for cid in range(N_CORES)
    ]


def assemble_out(results, bo):
    out = np.empty((GROUPS, WS, NWJ, WS, C), dtype=np.float32)
    for cid in range(N_CORES):
        r = np.asarray(results[cid]["out"], dtype=np.float32)
        # [gp, (g2,p,q), (j,Cout)] -> [gp, g2, p, j, q, c] -> [16 groups, p, j, q, c]
        r = r.reshape(GPAIRS, 2, WS, WS, NWJ, C).transpose(0, 1, 2, 4, 3, 5)
        out[cid * G_PER_CORE : (cid + 1) * G_PER_CORE] = r.reshape(
            G_PER_CORE, WS, NWJ, WS, C
        )
    out = out.reshape(B, H, WD, C)
    bo = np.asarray(bo, dtype=np.float32)
    if np.any(bo):
        out = out + bo
    return out


def run(x, attn, Wv, Wo, bo, dt_mm=None, **spmd_kwargs):
    nc = get_nc()
    in_maps = make_in_maps(x, attn, Wv, Wo)
    res = run_bass_kernel_spmd(nc, in_maps, core_ids=list(range(N_CORES)), **spmd_kwargs)
    return assemble_out(res.results, bo), res


def kernel(x, attn, Wv, Wo, bo):
    out, _ = run(x, attn, Wv, Wo, bo)
    return out
